# revision 1
# baseline (speedup 1.0000x reference)
"""Trainium2 Bass kernel for nn_EdgeConvolution (gnn_message_passing).

Math
----
Reference (B=2, N=512, C=128, U=128), adj binary {0,1}:
  masked[b,i,j,:]  = adj[b,i,j] * x[b,i,:]
  a_sel[b,i]       = adj[b,i, xidx[b,i]]
  edging[b,i,j,:]  = [ adj*x_i | adj*(a_sel - adj)*x_i ]
                   = adj[b,i,j] * [ x_i | (a_sel_i - 1)*x_i ]        (adj^2 = adj)
  out[b,i,j,:]     = relu(adj*(u_i + (a_sel_i-1)*v_i) + b),  u = x@W1, v = x@W2
So over j there are only two values per (b,i):
  z1_i = relu(u_i + (a_sel_i-1)*v_i + b)   (edges with adj=1, count k_i)
  z0   = relu(b)                            (edges with adj=0, count N-k_i)
  maxp_i   = max(1[k_i>0]*z1_i, 1[k_i<N]*z0)
  n_i      = k_i*1[any z1_i>0] + (N-k_i)*1[any z0>0]
  avgpool_i = [ k_i*x_i | k_i*(a_sel_i-1)*x_i ] / n_i
Per-core slab: 128 of the 1024 (b,i) rows; w/b replicated.
"""

import numpy as np

B, N, C, U = 2, 512, 128, 128
P = 128          # rows (b,i) per core == SBUF partitions
NCORES = 8
OUTF = U + 2 * C  # 384

_CACHE: dict = {}


def _build_nc():
    import concourse.bacc as bacc
    import concourse.bass as bass
    import concourse.mybir as mybir
    import concourse.tile as tile
    from concourse.masks import make_identity

    f32 = mybir.dt.float32
    i32 = mybir.dt.int32
    Alu = mybir.AluOpType
    AX = mybir.AxisListType.X

    nc = bacc.Bacc("TRN2", target_bir_lowering=False, debug=False,
                   num_devices=NCORES)

    adj_d = nc.dram_tensor("adj", [P, N], f32, kind="ExternalInput")
    x_d = nc.dram_tensor("x", [P, C], f32, kind="ExternalInput")
    xidx_d = nc.dram_tensor("xidx", [P, 1], i32, kind="ExternalInput")
    w_d = nc.dram_tensor("w", [2 * C, U], f32, kind="ExternalInput")
    b_d = nc.dram_tensor("b", [1, U], f32, kind="ExternalInput")
    out_d = nc.dram_tensor("out", [P, OUTF], f32, kind="ExternalOutput")

    with tile.TileContext(nc) as tc:
        with tc.tile_pool(name="sb", bufs=1) as pool, \
             tc.tile_pool(name="ps", bufs=1, space="PSUM") as psum:
            # ---- constants ----
            iota_f = pool.tile([P, N], f32)
            nc.gpsimd.iota(iota_f[:], pattern=[[1, N]], base=0,
                           channel_multiplier=0,
                           allow_small_or_imprecise_dtypes=True)
            ident = pool.tile([P, P], f32)
            make_identity(nc, ident[:])

            wcat = pool.tile([P, 2 * U], f32)  # [c, U|U] = [W1 | W2]
            nc.sync.dma_start(out=wcat[:, 0:U], in_=w_d.ap()[0:C, :])
            nc.sync.dma_start(out=wcat[:, U:2 * U], in_=w_d.ap()[C:2 * C, :])

            bfull = pool.tile([P, U], f32)     # b broadcast to all partitions
            b_ap = b_d.ap()
            nc.sync.dma_start(
                out=bfull[:],
                in_=bass.AP(b_ap.tensor, b_ap.offset, [[0, P], [1, U]]))

            # ---- inputs ----
            adj_t = pool.tile([P, N], f32)
            nc.sync.dma_start(out=adj_t[:], in_=adj_d.ap())
            x_t = pool.tile([P, C], f32)
            nc.sync.dma_start(out=x_t[:], in_=x_d.ap())
            xidx_i = pool.tile([P, 1], i32)
            nc.sync.dma_start(out=xidx_i[:], in_=xidx_d.ap())
            xidx_f = pool.tile([P, 1], f32)
            nc.gpsimd.tensor_copy(xidx_f[:], xidx_i[:])

            # ---- row stats: k = sum_j adj, a_sel = adj[i, xidx_i] ----
            k = pool.tile([P, 1], f32)
            nc.vector.reduce_sum(k[:], adj_t[:], axis=AX)
            scr = pool.tile([P, N], f32)
            a_sel = pool.tile([P, 1], f32)
            nc.vector.scalar_tensor_tensor(
                out=scr[:], in0=iota_f[:], scalar=xidx_f[:, 0:1], in1=adj_t[:],
                op0=Alu.is_equal, op1=Alu.mult, accum_out=a_sel[:, 0:1])

            # ---- u|v = x @ [W1|W2] (PE) ----
            xT_ps = psum.tile([P, P], f32)
            nc.tensor.transpose(xT_ps[:], x_t[:], ident[:])
            xT = pool.tile([P, P], f32)
            nc.vector.tensor_copy(xT[:], xT_ps[:])
            mm = psum.tile([P, 2 * U], f32)    # [i, u | v]
            nc.tensor.matmul(mm[:], lhsT=xT[:], rhs=wcat[:], start=True,
                             stop=True)

            # ---- z0 = relu(b), s0 = any(z0 > 0) ----
            z0 = pool.tile([P, U], f32)
            z0sum = pool.tile([P, 1], f32)
            nc.vector.tensor_scalar(out=z0[:], in0=bfull[:], scalar1=0.0,
                                    scalar2=None, op0=Alu.max, op1=Alu.add,
                                    accum_out=z0sum[:, 0:1])
            s0 = pool.tile([P, 1], f32)
            nc.vector.tensor_scalar(out=s0[:], in0=z0sum[:], scalar1=0.0,
                                    scalar2=None, op0=Alu.is_gt)

            # ---- z1 = relu(u + (a_sel-1)*v + b), s1 = any(z1 > 0) ----
            asm1 = pool.tile([P, 1], f32)
            nc.vector.tensor_scalar(out=asm1[:], in0=a_sel[:], scalar1=-1.0,
                                    scalar2=None, op0=Alu.add)
            upb = pool.tile([P, U], f32)
            nc.vector.tensor_add(upb[:], mm[:, 0:U], bfull[:])
            zz = pool.tile([P, U], f32)
            nc.vector.scalar_tensor_tensor(
                out=zz[:], in0=mm[:, U:2 * U], scalar=asm1[:, 0:1],
                in1=upb[:], op0=Alu.mult, op1=Alu.add)
            z1 = pool.tile([P, U], f32)
            z1sum = pool.tile([P, 1], f32)
            nc.vector.tensor_scalar(out=z1[:], in0=zz[:], scalar1=0.0,
                                    scalar2=None, op0=Alu.max, op1=Alu.add,
                                    accum_out=z1sum[:, 0:1])
            s1 = pool.tile([P, 1], f32)
            nc.vector.tensor_scalar(out=s1[:], in0=z1sum[:], scalar1=0.0,
                                    scalar2=None, op0=Alu.is_gt)

            # ---- n = k*s1 + (N-k)*s0 ; c1 = k/n ; c2 = c1*(a_sel-1) ----
            h1 = pool.tile([P, 1], f32)
            nc.vector.tensor_scalar(out=h1[:], in0=k[:], scalar1=0.0,
                                    scalar2=None, op0=Alu.is_gt)
            h0 = pool.tile([P, 1], f32)
            nc.vector.tensor_scalar(out=h0[:], in0=k[:], scalar1=float(N),
                                    scalar2=None, op0=Alu.is_lt)
            nk = pool.tile([P, 1], f32)
            nc.vector.tensor_scalar(out=nk[:], in0=k[:], scalar1=-1.0,
                                    scalar2=float(N), op0=Alu.mult,
                                    op1=Alu.add)
            t2 = pool.tile([P, 1], f32)
            nc.vector.tensor_mul(t2[:], nk[:], s0[:])
            n_t = pool.tile([P, 1], f32)
            nc.vector.scalar_tensor_tensor(
                out=n_t[:], in0=k[:], scalar=s1[:, 0:1], in1=t2[:],
                op0=Alu.mult, op1=Alu.add)
            rn = pool.tile([P, 1], f32)
            nc.vector.reciprocal(rn[:], n_t[:])
            c1 = pool.tile([P, 1], f32)
            nc.vector.tensor_mul(c1[:], k[:], rn[:])
            c2 = pool.tile([P, 1], f32)
            nc.vector.tensor_mul(c2[:], c1[:], asm1[:])

            # ---- assemble output [maxp | k*x/n | k*(a_sel-1)*x/n] ----
            out_t = pool.tile([P, OUTF], f32)
            z0h = pool.tile([P, U], f32)
            nc.vector.tensor_scalar_mul(z0h[:], z0[:], h0[:, 0:1])
            nc.vector.scalar_tensor_tensor(
                out=out_t[:, 0:U], in0=z1[:], scalar=h1[:, 0:1], in1=z0h[:],
                op0=Alu.mult, op1=Alu.max)
            nc.vector.tensor_scalar_mul(out_t[:, U:U + C], x_t[:], c1[:, 0:1])
            nc.vector.tensor_scalar_mul(out_t[:, U + C:OUTF], x_t[:],
                                        c2[:, 0:1])
            nc.sync.dma_start(out=out_d.ap(), in_=out_t[:])

    nc.compile()
    return nc


def get_nc():
    if "nc" not in _CACHE:
        _CACHE["nc"] = _build_nc()
    return _CACHE["nc"]


def make_in_maps(inputs, adj_matrix, xidx, w, b):
    """Shard full inputs into per-core input maps (128 (b,i) rows per core)."""
    x_flat = np.ascontiguousarray(
        np.asarray(inputs, dtype=np.float32).reshape(B * N, C))
    adj_flat = np.ascontiguousarray(
        np.asarray(adj_matrix, dtype=np.float32).reshape(B * N, N))
    xidx_flat = np.ascontiguousarray(
        np.asarray(xidx, dtype=np.int32).reshape(B * N, 1))
    w_full = np.ascontiguousarray(np.asarray(w, dtype=np.float32)[0])
    b_full = np.ascontiguousarray(
        np.asarray(b, dtype=np.float32).reshape(1, U))

    in_maps = []
    for c in range(NCORES):
        rows = slice(c * P, (c + 1) * P)
        in_maps.append({
            "adj": adj_flat[rows],
            "x": x_flat[rows],
            "xidx": xidx_flat[rows],
            "w": w_full,
            "b": b_full,
        })
    return in_maps


def kernel(inputs, adj_matrix, xidx, w, b, _trace=False):
    from concourse.bass_utils import run_bass_kernel_spmd

    nc = get_nc()
    in_maps = make_in_maps(inputs, adj_matrix, xidx, w, b)
    res = run_bass_kernel_spmd(nc, in_maps, list(range(NCORES)),
                               trace=_trace)
    out = np.concatenate([res.results[c]["out"] for c in range(NCORES)],
                         axis=0)
    out = out.reshape(B, N, OUTF).astype(np.float32)
    if _trace:
        _CACHE["last_results"] = res
    return out
